# revision 24
# baseline (speedup 1.0000x reference)
"""GATv2 4-layer stack on 8 TRN2 NeuronCores (bass/Tile, SPMD).

Strategy (edge-parallel, dst-sharded):
- Edges sorted by dst, sharded across 8 cores by dst-node range (6250/core).
- Dst nodes packed into variable windows (<=126 nodes, <=16 blocks of 128
  edge slots). Window structure shared across cores (per-window slot counts
  are the max over cores, padded to 128; pad slots duplicate a real index
  and scatter to trash row 127).
- Per layer: xT via one DMA-transpose; dense xl/xr = x @ [Wl|Wr] on PE;
  AllGather xl shards -> full 51200-row table in every core's HBM.
- Gathers of xl[src] (edge-major bf16, 256B rows) batched as 2 calls per
  6-window group, round-robin over 4 SWDGE queues (Q7 core pairs overlap).
- Per window: b_em one-hot via DVE is_equal; b_fm via PE transposes + ACT
  copy; s^T built in PSUM as transpose(xl_em) + one-hot xr expansion
  (PSUM-accumulated); ACT Prelu(0.2); per-block score matmuls (att as rhs);
  ACT exp into the [w|exp] tile; DVE w = xl*exp; per-block aggregation
  matmul with rhs=[w|exp] yields out and den together; normalize + bias
  (+relu) and store.
"""
import numpy as np

N = 50000
E = 800000
F = 128
HID = 64
NEG_SLOPE = 0.2
NCORES = 8
NPC = N // NCORES            # 6250
NBLK = 50
NPAD = NBLK * 128            # 6400
TBL = NPAD * NCORES          # 51200
LO_LIM = 32768
HEADS = [2, 2, 2, 1]
WCAP = 126                   # max dst nodes per window (row 127 = trash)
EBCAP = 16                   # max edge blocks (128 slots) per window
GRP = 6                      # windows per gather group
QUEUES = 4                   # SWDGE queues used
BPC = 4                      # blocks per score chunk


def _pad128(n):
    return max(128, (int(n) + 127) // 128 * 128)


def _wrap_idx(flat):
    # [n] -> [128, n//16] int16: slot q at [q%16, q//16], tiled over 8 groups
    n = flat.shape[0]
    w = flat.reshape(n // 16, 16).T.astype(np.int16)
    return np.tile(w, (8, 1))


def _plan(deg_lo, deg_hi):
    """Shared window structure: list of (n0, n1, EL, EH) with EL/EH the
    max-over-cores padded slot counts."""
    windows = []
    n0 = 0
    cl = np.zeros(NCORES, np.int64)
    ch = np.zeros(NCORES, np.int64)
    for v in range(NPC):
        nl = cl + deg_lo[:, v]
        nh = ch + deg_hi[:, v]
        blocks = (_pad128(nl.max()) + _pad128(nh.max())) // 128
        if v > n0 and (v - n0 + 1 > WCAP or blocks > EBCAP):
            windows.append((n0, v, _pad128(cl.max()), _pad128(ch.max())))
            n0 = v
            cl = deg_lo[:, v].copy()
            ch = deg_hi[:, v].copy()
        else:
            cl, ch = nl, nh
    windows.append((n0, NPC, _pad128(cl.max()), _pad128(ch.max())))
    return windows


def _prep(inputs):
    import ml_dtypes
    bf16 = ml_dtypes.bfloat16
    x = np.asarray(inputs["x"], dtype=np.float32)
    ei = np.asarray(inputs["edge_index"]).astype(np.int64)
    src, dst = ei[0], ei[1]

    order = np.argsort(dst, kind="stable")
    src, dst = src[order], dst[order]
    prow = (src // NPC) * NPAD + (src % NPC)
    core_of = dst // NPC
    dloc = dst % NPC
    lo_mask = prow < LO_LIM

    deg_lo = np.zeros((NCORES, NPC), np.int64)
    deg_hi = np.zeros((NCORES, NPC), np.int64)
    np.add.at(deg_lo, (core_of, dloc), lo_mask)
    np.add.at(deg_hi, (core_of, dloc), ~lo_mask)

    windows = _plan(deg_lo, deg_hi)
    NW = len(windows)
    groups = [list(range(g, min(g + GRP, NW))) for g in range(0, NW, GRP)]

    # per-core edge lists per (window, lo/hi), in dst-sorted order
    # edge ranges per (core, node) via cumulative counts
    start_of_core = np.searchsorted(core_of, np.arange(NCORES))
    # edges per core are contiguous; within, sorted by dloc
    ebuf_lo = [[] for _ in range(NCORES)]
    ebuf_hi = [[] for _ in range(NCORES)]
    node_ptr = np.zeros((NCORES, NPC + 1), np.int64)
    for c in range(NCORES):
        s0 = start_of_core[c]
        s1 = start_of_core[c + 1] if c + 1 < NCORES else E
        cnt = np.bincount(dloc[s0:s1], minlength=NPC)
        node_ptr[c, 1:] = np.cumsum(cnt) + s0
        node_ptr[c, 0] = s0

    # layout: per window w: EL slots (lo) then EH slots (hi)
    EBTOT = sum((el + eh) // 128 for (_, _, el, eh) in windows)
    LOTOT = sum(el for (_, _, el, _) in windows)
    HITOT = sum(eh for (_, _, _, eh) in windows)

    idxlo = np.zeros((NCORES, 128, LOTOT // 16), np.int16)
    idxhi = np.zeros((NCORES, 128, HITOT // 16), np.int16)
    dstcol = np.zeros((NCORES, 128, EBTOT), bf16)

    lo_off = 0
    hi_off = 0
    col_off = 0
    meta = []  # per window: (n0, n1, EL, EH, col_off)
    for (n0, n1, el, eh) in windows:
        meta.append((n0, n1, el, eh, col_off))
        for c in range(NCORES):
            s0, s1 = node_ptr[c, n0], node_ptr[c, n1]
            pr = prow[s0:s1]
            dl = dloc[s0:s1] - n0
            m = pr < LO_LIM
            flat_lo = np.zeros(el, np.int64)
            nlo = int(m.sum())
            flat_lo[:nlo] = pr[m]
            flat_lo[nlo:] = flat_lo[max(nlo - 1, 0)]
            flat_hi = np.zeros(eh, np.int64)
            nhi = int((~m).sum())
            flat_hi[:nhi] = pr[~m] - LO_LIM
            flat_hi[nhi:] = flat_hi[max(nhi - 1, 0)]
            idxlo[c, :, lo_off // 16:(lo_off + el) // 16] = _wrap_idx(flat_lo)
            idxhi[c, :, hi_off // 16:(hi_off + eh) // 16] = _wrap_idx(flat_hi)
            rows = np.full(el + eh, 127, np.int64)
            rows[:nlo] = dl[m]
            rows[el:el + nhi] = dl[~m]
            eb = (el + eh) // 128
            dstcol[c, :, col_off:col_off + eb] = rows.reshape(eb, 128).T.astype(bf16)
        lo_off += el
        hi_off += eh
        col_off += (el + eh) // 128

    # x shards, padded + cast
    xsh = np.zeros((NCORES, NPAD, F), bf16)
    xv = x.astype(bf16)
    for c in range(NCORES):
        xsh[c, :NPC] = xv[c * NPC:(c + 1) * NPC]

    wboth = np.zeros((4, F, 256), bf16)
    attm = np.zeros((4, F, 2), bf16)
    bias = np.zeros((4, 1, F), np.float32)
    for L in range(4):
        Wl = np.asarray(inputs[f"Wl{L + 1}"], np.float32)
        Wr = np.asarray(inputs[f"Wr{L + 1}"], np.float32)
        att = np.asarray(inputs[f"att{L + 1}"], np.float32)
        b = np.asarray(inputs[f"b{L + 1}"], np.float32)
        fo = Wl.shape[1]
        wboth[L, :, :fo] = Wl.astype(bf16)
        wboth[L, :, 128:128 + fo] = Wr.astype(bf16)
        for h in range(att.shape[0]):
            attm[L, h * HID:(h + 1) * HID, h] = att[h].astype(bf16)
        bias[L, 0, :fo] = b

    ident = np.eye(128, dtype=bf16)
    iota_row = np.arange(128, dtype=np.float32).astype(bf16).reshape(1, 128)
    zeros = np.zeros((NPAD - NPC, F), bf16)

    per_core = []
    for c in range(NCORES):
        per_core.append({
            "x0": xsh[c], "idxlo": idxlo[c], "idxhi": idxhi[c],
            "dstcol": dstcol[c], "wboth": wboth, "attm": attm, "bias": bias,
            "ident": ident, "iota": iota_row, "zeros": zeros,
        })
    plan = (tuple(meta), tuple(tuple(g) for g in groups), LOTOT, HITOT, EBTOT)
    return per_core, plan


# ---------------------------------------------------------------- program

def build_program(plan, n_layers=4, debug=False):
    import concourse.bass as bass
    import concourse.mybir as mybir
    import concourse.tile as tile
    from concourse import bacc

    meta, groups, LOTOT, HITOT, EBTOT = plan
    EBMAX = max((el + eh) // 128 for (_, _, el, eh, _) in meta)
    # per-group slot counts
    gmeta = []
    lo_off = 0
    hi_off = 0
    for g in groups:
        nlo = sum(meta[w][2] for w in g)
        nhi = sum(meta[w][3] for w in g)
        gmeta.append((lo_off, nlo, hi_off, nhi))
        lo_off += nlo
        hi_off += nhi
    NBGMAX = max((nlo + nhi) // 128 for (_, nlo, _, nhi) in gmeta)

    dt = mybir.dt
    nc = bacc.Bacc(None, num_swdge_queues=4)

    x0 = nc.dram_tensor("x0", [NPAD, F], dt.bfloat16, kind="ExternalInput")
    idxlo_d = nc.dram_tensor("idxlo", [128, LOTOT // 16], dt.int16, kind="ExternalInput")
    idxhi_d = nc.dram_tensor("idxhi", [128, HITOT // 16], dt.int16, kind="ExternalInput")
    dstcol_d = nc.dram_tensor("dstcol", [128, EBTOT], dt.bfloat16, kind="ExternalInput")
    wboth_d = nc.dram_tensor("wboth", [4, F, 256], dt.bfloat16, kind="ExternalInput")
    attm_d = nc.dram_tensor("attm", [4, F, 2], dt.bfloat16, kind="ExternalInput")
    bias_d = nc.dram_tensor("bias", [4, 1, F], dt.float32, kind="ExternalInput")
    ident_d = nc.dram_tensor("ident", [128, 128], dt.bfloat16, kind="ExternalInput")
    iota_d = nc.dram_tensor("iota", [1, 128], dt.bfloat16, kind="ExternalInput")
    zeros_d = nc.dram_tensor("zeros", [NPAD - NPC, F], dt.bfloat16, kind="ExternalInput")

    out_d = nc.dram_tensor("out", [NPAD, HID], dt.float32, kind="ExternalOutput")
    if debug:
        eb0 = (meta[0][2] + meta[0][3]) // 128
        nbg0 = (gmeta[0][1] + gmeta[0][3]) // 128
        dbg = {
            "d_xT": nc.dram_tensor("d_xT", [128, NPAD], dt.bfloat16, kind="ExternalOutput"),
            "d_xl": nc.dram_tensor("d_xl", [128, F], dt.bfloat16, kind="ExternalOutput"),
            "d_xr": nc.dram_tensor("d_xr", [128, F], dt.bfloat16, kind="ExternalOutput"),
            "d_xlem": nc.dram_tensor("d_xlem", [128, nbg0 * F], dt.bfloat16, kind="ExternalOutput"),
            "d_bem": nc.dram_tensor("d_bem", [128, eb0 * 128], dt.bfloat16, kind="ExternalOutput"),
            "d_bfm": nc.dram_tensor("d_bfm", [128, eb0 * 128], dt.bfloat16, kind="ExternalOutput"),
            "d_lk": nc.dram_tensor("d_lk", [128, 512], dt.bfloat16, kind="ExternalOutput"),
            "d_sc": nc.dram_tensor("d_sc", [128, 48], dt.float32, kind="ExternalOutput"),
            "d_wexp": nc.dram_tensor("d_wexp", [128, eb0 * 144], dt.bfloat16, kind="ExternalOutput"),
            "d_od": nc.dram_tensor("d_od", [128, 136], dt.float32, kind="ExternalOutput"),
            "d_ps": nc.dram_tensor("d_ps", [128, 512], dt.float32, kind="ExternalOutput"),
            "d_xrw": nc.dram_tensor("d_xrw", [128, F], dt.bfloat16, kind="ExternalOutput"),
        }

    xl_loc = nc.dram_tensor("xl_loc", [NPAD, F], dt.bfloat16)
    xr_loc = nc.dram_tensor("xr_loc", [NPAD, F], dt.bfloat16)
    xl_all = nc.dram_tensor("xl_all", [TBL, F], dt.bfloat16, addr_space="Shared")
    x_cur = nc.dram_tensor("x_cur", [NPAD, F], dt.bfloat16)

    replica = [list(range(NCORES))]
    qrr = [0]  # round-robin queue counter

    with tile.TileContext(nc) as tc:
        cpool = tc.alloc_tile_pool(name="cpool", bufs=1)
        spool = tc.alloc_tile_pool(name="spool", bufs=3)
        gpool = tc.alloc_tile_pool(name="gpool", bufs=2)
        wpool = tc.alloc_tile_pool(name="wpool", bufs=2)
        ppool = tc.alloc_tile_pool(name="ppool", bufs=2, space="PSUM")
        papool = tc.alloc_tile_pool(name="papool", bufs=2, space="PSUM")
        bfpool = tc.alloc_tile_pool(name="bfpool", bufs=1, space="PSUM")

        ident_sb = cpool.tile([128, 128], dt.bfloat16)
        nc.sync.dma_start(out=ident_sb[:], in_=ident_d[:])
        iota_sb = cpool.tile([128, 128], dt.bfloat16)
        nc.sync.dma_start(out=iota_sb[:], in_=iota_d[:].to_broadcast([128, 128]))

        for L in range(n_layers):
            H = HEADS[L]
            C = F // H
            last = L == n_layers - 1
            wb_sb = cpool.tile([F, 256], dt.bfloat16, tag="wb")
            nc.sync.dma_start(out=wb_sb[:], in_=wboth_d[L])
            attm_sb = cpool.tile([F, 2], dt.bfloat16, tag="attm")
            nc.sync.dma_start(out=attm_sb[:], in_=attm_d[L])
            bias_sb = cpool.tile([128, F], dt.float32, tag="bias")
            nc.sync.dma_start(out=bias_sb[:], in_=bias_d[L].to_broadcast([128, F]))

            xin = x0 if L == 0 else x_cur

            # ---- dense: xT via DMA transpose, then xl/xr = x @ [Wl|Wr]
            xT_all = gpool.tile([128, NPAD], dt.bfloat16, tag="xT")
            nc.sync.dma_start(out=xT_all[:], in_=xin[:], transpose=True)
            if debug and L == 0:
                nc.sync.dma_start(out=dbg["d_xT"][:], in_=xT_all[:])
            for blk in range(NBLK):
                plr = ppool.tile([128, 512], dt.float32, tag="schunk", space="PSUM")
                nc.tensor.matmul(out=plr[:, 0:256],
                                 lhsT=xT_all[:, blk * 128:(blk + 1) * 128],
                                 rhs=wb_sb[:], start=True, stop=True)
                xlb = spool.tile([128, F], dt.bfloat16, tag="xlb")
                nc.scalar.activation(out=xlb[:], in_=plr[:, 0:128],
                                     func=mybir.ActivationFunctionType.Copy)
                xrb = spool.tile([128, F], dt.bfloat16, tag="xrb")
                nc.vector.tensor_copy(out=xrb[:], in_=plr[:, 128:256])
                nc.sync.dma_start(out=xl_loc[blk * 128:(blk + 1) * 128, :], in_=xlb[:])
                nc.sync.dma_start(out=xr_loc[blk * 128:(blk + 1) * 128, :], in_=xrb[:])
                if debug and L == 0 and blk == 0:
                    nc.sync.dma_start(out=dbg["d_xl"][:], in_=xlb[:])
                    nc.sync.dma_start(out=dbg["d_xr"][:], in_=xrb[:])

            # ---- allgather xl shards -> full table
            nc.gpsimd.collective_compute(
                "AllGather", mybir.AluOpType.bypass, replica_groups=replica,
                ins=[xl_loc[:]], outs=[xl_all[:]],
            )

            # ---- window groups
            for gi, g in enumerate(groups):
                glo_off, gnlo, ghi_off, gnhi = gmeta[gi]
                nbg_lo = gnlo // 128
                nbg = (gnlo + gnhi) // 128
                ilo = spool.tile([128, max(g2[1] for g2 in gmeta) // 16],
                                 dt.int16, tag="ilo")
                nc.sync.dma_start(out=ilo[:, 0:gnlo // 16],
                                  in_=idxlo_d[:, glo_off // 16:(glo_off + gnlo) // 16])
                ihi = spool.tile([128, max(g2[3] for g2 in gmeta) // 16],
                                 dt.int16, tag="ihi")
                nc.sync.dma_start(out=ihi[:, 0:gnhi // 16],
                                  in_=idxhi_d[:, ghi_off // 16:(ghi_off + gnhi) // 16])

                xl_em = gpool.tile([128, NBGMAX, F], dt.bfloat16, tag="xlem")
                nc.gpsimd.dma_gather(
                    out_ap=xl_em[:, 0:nbg_lo, :], in_ap=xl_all[0:LO_LIM, :],
                    idxs_ap=ilo[:, 0:gnlo // 16], num_idxs=gnlo,
                    num_idxs_reg=gnlo, elem_size=F, transpose=False,
                    single_packet=False, queue_num=(qrr[0] % 4) if QUEUES > 1 else 0)
                qrr[0] += 1
                nc.gpsimd.dma_gather(
                    out_ap=xl_em[:, nbg_lo:nbg, :], in_ap=xl_all[LO_LIM:TBL, :],
                    idxs_ap=ihi[:, 0:gnhi // 16], num_idxs=gnhi,
                    num_idxs_reg=gnhi, elem_size=F, transpose=False,
                    single_packet=False, queue_num=(qrr[0] % 4) if QUEUES > 1 else 0)
                qrr[0] += 1

                blk_lo = 0
                blk_hi = nbg_lo
                for w in g:
                    n0, n1, el, eh, col_off = meta[w]
                    ebw = (el + eh) // 128
                    nlo_b = el // 128
                    nhi_b = eh // 128
                    # window block indices in xl_em
                    wblocks = list(range(blk_lo, blk_lo + nlo_b)) + \
                        list(range(blk_hi, blk_hi + nhi_b))
                    blk_lo += nlo_b
                    blk_hi += nhi_b

                    dcol = spool.tile([128, EBMAX], dt.bfloat16, tag="dcol")
                    nc.sync.dma_start(out=dcol[:, 0:ebw],
                                      in_=dstcol_d[:, col_off:col_off + ebw])
                    xr_win = spool.tile([128, F], dt.bfloat16, tag="xrwin")
                    nc.sync.dma_start(out=xr_win[:], in_=xr_loc[n0:n0 + 128, :])

                    # one-hots
                    b_em = wpool.tile([128, EBMAX, 128], dt.bfloat16, tag="bem")
                    nc.vector.tensor_tensor(
                        out=b_em[:, 0:ebw, :],
                        in0=dcol[:, 0:ebw, None].to_broadcast([128, ebw, 128]),
                        in1=iota_sb[:].unsqueeze(1).to_broadcast([128, ebw, 128]),
                        op=mybir.AluOpType.is_equal)
                    bfm_ps = bfpool.tile([128, EBMAX * 128], dt.bfloat16,
                                         tag="bfm", space="PSUM")
                    for j in range(ebw):
                        nc.tensor.matmul(out=bfm_ps[:, j * 128:(j + 1) * 128],
                                         lhsT=b_em[:, j, :], rhs=ident_sb[:],
                                         is_transpose=True, start=True, stop=True,
                                         skip_group_check=True)
                    b_fm = wpool.tile([128, EBMAX * 128], dt.bfloat16, tag="bfmsb")
                    nc.scalar.activation(out=b_fm[:, 0:ebw * 128],
                                         in_=bfm_ps[:, 0:ebw * 128],
                                         func=mybir.ActivationFunctionType.Copy)
                    if debug and L == 0 and w == 0:
                        nc.sync.dma_start(
                            out=dbg["d_xlem"][:],
                            in_=xl_em[:, 0:(gnlo + gnhi) // 128, :].rearrange(
                                "p b f -> p (b f)"))
                        nc.sync.dma_start(out=dbg["d_bem"][:],
                                          in_=b_em[:, 0:ebw, :].rearrange(
                                              "p b f -> p (b f)"))
                        nc.sync.dma_start(out=dbg["d_bfm"][:],
                                          in_=b_fm[:, 0:ebw * 128])

                    # scores (+ wexp tile holds w and exp)
                    sd_ps = papool.tile([128, 512], dt.float32, tag="sd",
                                        space="PSUM")
                    wexp = wpool.tile([128, EBMAX, 144], dt.bfloat16, tag="wexp")
                    for q in range((ebw + BPC - 1) // BPC):
                        k0 = q * BPC
                        k1 = min(ebw, k0 + BPC)
                        cw = (k1 - k0) * 128
                        ps = ppool.tile([128, 512], dt.float32, tag="schunk",
                                        space="PSUM")
                        for k in range(k0, k1):
                            # xl^T block via regular matmul (identity rhs),
                            # then xr one-hot expansion accumulates into the
                            # SAME region (proper accumulation group).
                            reg = ps[:, (k - k0) * 128:(k - k0 + 1) * 128]
                            nc.tensor.matmul(
                                out=reg,
                                lhsT=xl_em[:, wblocks[k], :], rhs=ident_sb[:],
                                start=True, stop=False,
                                skip_group_check=True)
                            nc.tensor.matmul(
                                out=reg, lhsT=xr_win[:],
                                rhs=b_fm[:, k * 128:(k + 1) * 128],
                                start=False, stop=True, skip_group_check=True)
                        lk = spool.tile([128, 512], dt.bfloat16, tag="lk")
                        nc.scalar.activation(out=lk[:, 0:cw], in_=ps[:, 0:cw],
                                             func=mybir.ActivationFunctionType.Prelu,
                                             alpha=NEG_SLOPE)
                        if debug and L == 0 and w == 0 and q == 0:
                            nc.sync.dma_start(out=dbg["d_lk"][:, 0:cw],
                                              in_=lk[:, 0:cw])
                            ps_sb = spool.tile([128, 512], dt.float32,
                                               tag="psdump")
                            nc.vector.tensor_copy(out=ps_sb[:, 0:cw],
                                                  in_=ps[:, 0:cw])
                            nc.sync.dma_start(out=dbg["d_ps"][:, 0:cw],
                                              in_=ps_sb[:, 0:cw])
                            nc.sync.dma_start(out=dbg["d_xrw"][:], in_=xr_win[:])
                        for k in range(k0, k1):
                            nc.tensor.matmul(
                                out=sd_ps[:, k * H:k * H + H],
                                lhsT=lk[:, (k - k0) * 128:(k - k0 + 1) * 128],
                                rhs=attm_sb[:, 0:H], start=True, stop=True,
                                skip_group_check=True)

                    # exp into wexp[:, :, 128:128+H]
                    nc.scalar.activation(
                        out=wexp[:, 0:ebw, 128:128 + H],
                        in_=sd_ps[:, 0:ebw * H].rearrange("p (b h) -> p b h", h=H),
                        func=mybir.ActivationFunctionType.Exp)

                    # w = xl * exp  (two ops: lo block range, hi block range)
                    for (b0, nb) in ((0, nlo_b), (nlo_b, nhi_b)):
                        if nb == 0:
                            continue
                        xsl = xl_em[:, wblocks[b0]:wblocks[b0] + nb, :]
                        nc.vector.tensor_tensor(
                            out=wexp[:, b0:b0 + nb, 0:128].rearrange(
                                "p b (h c) -> p b h c", h=H),
                            in0=xsl.rearrange("p b (h c) -> p b h c", h=H),
                            in1=wexp[:, b0:b0 + nb, 128:128 + H, None
                                     ].to_broadcast([128, nb, H, C]),
                            op=mybir.AluOpType.mult)

                    if debug and L == 0 and w == 0:
                        sc_sb = spool.tile([128, 48], dt.float32, tag="scdump")
                        nc.vector.tensor_copy(out=sc_sb[:, 0:ebw * H],
                                              in_=sd_ps[:, 0:ebw * H])
                        nc.sync.dma_start(out=dbg["d_sc"][:, 0:ebw * H],
                                          in_=sc_sb[:, 0:ebw * H])

                    # aggregation: out | den
                    od_ps = sd_ps[:, 256:256 + F + H]
                    for j in range(ebw):
                        nc.tensor.matmul(out=od_ps[:],
                                         lhsT=b_em[:, j, :],
                                         rhs=wexp[:, j, 0:F + H],
                                         start=(j == 0), stop=(j == ebw - 1),
                                         skip_group_check=True)
                    if debug and L == 0 and w == 0:
                        nc.sync.dma_start(out=dbg["d_wexp"][:],
                                          in_=wexp[:, 0:ebw, :].rearrange(
                                              "p b f -> p (b f)"))
                        od_sb = spool.tile([128, 136], dt.float32, tag="oddump")
                        nc.vector.tensor_copy(out=od_sb[:, 0:F + H],
                                              in_=od_ps[:])
                        nc.sync.dma_start(out=dbg["d_od"][:, 0:F + H],
                                          in_=od_sb[:, 0:F + H])

                    # normalize + bias (+ relu)
                    den = spool.tile([128, 2], dt.float32, tag="den")
                    nc.vector.tensor_scalar_add(den[:, 0:H], od_ps[:, F:F + H],
                                                1e-20)
                    rec = spool.tile([128, 2], dt.float32, tag="rec")
                    nc.vector.reciprocal(rec[:, 0:H], den[:, 0:H])
                    xo = spool.tile([128, F], dt.float32, tag="xo")
                    for h in range(H):
                        nc.scalar.activation(out=xo[:, h * C:(h + 1) * C],
                                             in_=od_ps[:, h * C:(h + 1) * C],
                                             func=mybir.ActivationFunctionType.Copy,
                                             scale=rec[:, h:h + 1])
                    nc.vector.tensor_tensor(out=xo[:], in0=xo[:], in1=bias_sb[:],
                                            op=mybir.AluOpType.add)
                    nw = n1 - n0
                    if not last:
                        xn = spool.tile([128, F], dt.bfloat16, tag="xn")
                        nc.vector.tensor_scalar_max(xn[:], xo[:], 0.0)
                        nc.sync.dma_start(out=x_cur[n0:n1, :], in_=xn[0:nw, :])
                    else:
                        nc.sync.dma_start(out=out_d[n0:n1, :],
                                          in_=xo[0:nw, 0:HID])

            if not last:
                zpad = spool.tile([128, F], dt.bfloat16, tag="zpad")
                nc.sync.dma_start(out=zpad[:], in_=zeros_d[0:128])
                nc.sync.dma_start(out=x_cur[NPC:NPC + 128, :], in_=zpad[:])
                nc.sync.dma_start(out=x_cur[NPC + 128:NPAD, :],
                                  in_=zpad[0:NPAD - NPC - 128, :])

        for p in (bfpool, papool, ppool, wpool, gpool, spool, cpool):
            p.release()

    nc.finalize()
    return nc


# ---------------------------------------------------------------- runner

_CACHE = {}


def _make_runner(nc, in_maps, n_cores):
    import jax
    from jax.sharding import Mesh, PartitionSpec
    from jax.experimental.shard_map import shard_map
    import concourse.mybir as mybir
    from concourse import bass2jax
    from concourse.bass2jax import _bass_exec_p, partition_id_tensor

    bass2jax.install_neuronx_cc_hook()
    partition_name = nc.partition_id_tensor.name if nc.partition_id_tensor else None
    in_names, out_names, out_avals, zero_outs = [], [], [], []
    for alloc in nc.m.functions[0].allocations:
        if not isinstance(alloc, mybir.MemoryLocationSet):
            continue
        name = alloc.memorylocations[0].name
        if alloc.kind == "ExternalInput":
            if name != partition_name:
                in_names.append(name)
        elif alloc.kind == "ExternalOutput":
            shape = tuple(alloc.tensor_shape)
            dtype = mybir.dt.np(alloc.dtype)
            out_names.append(name)
            out_avals.append(jax.core.ShapedArray(shape, dtype))
            zero_outs.append(np.zeros(shape, dtype))
    n_params = len(in_names)
    n_outs = len(out_avals)
    in_names_all = in_names + out_names
    if partition_name is not None:
        in_names_all.append(partition_name)

    def _body(*args):
        operands = list(args)
        if partition_name is not None:
            operands.append(partition_id_tensor())
        outs = _bass_exec_p.bind(
            *operands, out_avals=tuple(out_avals), in_names=tuple(in_names_all),
            out_names=tuple(out_names), lowering_input_output_aliases=(),
            sim_require_finite=False, sim_require_nnan=False, nc=nc)
        return tuple(outs)

    devices = jax.devices()[:n_cores]
    mesh = Mesh(np.asarray(devices), ("core",))
    in_specs = (PartitionSpec("core"),) * (n_params + n_outs)
    out_specs = (PartitionSpec("core"),) * n_outs
    donate = tuple(range(n_params, n_params + n_outs))
    fn = jax.jit(shard_map(_body, mesh=mesh, in_specs=in_specs,
                           out_specs=out_specs, check_rep=False),
                 donate_argnums=donate, keep_unused=True)
    sharding = jax.sharding.NamedSharding(mesh, PartitionSpec("core"))
    concat_in = [
        jax.device_put(
            np.concatenate([np.asarray(in_maps[c][name]) for c in range(n_cores)],
                           axis=0), sharding)
        for name in in_names
    ]
    zero_big = [np.concatenate([z] * n_cores, axis=0) for z in zero_outs]

    def run():
        import jax as _jax
        outs = fn(*concat_in,
                  *[_jax.device_put(z, sharding) for z in zero_big])
        outs = [np.asarray(o) for o in outs]
        per_core = []
        for c in range(n_cores):
            m = {}
            for i, name in enumerate(out_names):
                rows = out_avals[i].shape[0]
                m[name] = outs[i][c * rows:(c + 1) * rows]
            per_core.append(m)
        return per_core

    return run


def kernel(**inputs) -> np.ndarray:
    per_core, plan = _prep(inputs)
    key = plan[0]
    if key not in _CACHE:
        nc = build_program(plan)
        _CACHE[key] = (nc, None)
    nc, runner = _CACHE[key]
    if runner is None:
        runner = _make_runner(nc, per_core, NCORES)
        _CACHE[key] = (nc, runner)
    res = runner()
    out = np.concatenate([res[c]["out"][:NPC] for c in range(NCORES)], axis=0)
    return out.astype(np.float32)


# revision 26
# speedup vs baseline: 1.3735x; 1.3735x over previous
"""GATv2 4-layer stack on 8 TRN2 NeuronCores (bass/Tile, SPMD).

Strategy (edge-parallel, dst-sharded):
- Edges sorted by dst, sharded across 8 cores by dst-node range (6250/core).
- Dst nodes packed into variable windows (<=126 nodes, <=16 blocks of 128
  edge slots). Window structure shared across cores (per-window slot counts
  are the max over cores, padded to 128; pad slots duplicate a real index
  and scatter to trash row 127).
- Per layer: xT via one DMA-transpose; dense xl/xr = x @ [Wl|Wr] on PE;
  AllGather xl shards -> full 51200-row table in every core's HBM.
- Gathers of xl[src] (edge-major bf16, 256B rows) batched as 2 calls per
  6-window group, round-robin over 4 SWDGE queues (Q7 core pairs overlap).
- Per window: b_em one-hot via DVE is_equal; b_fm via PE transposes + ACT
  copy; s^T built in PSUM as transpose(xl_em) + one-hot xr expansion
  (PSUM-accumulated); ACT Prelu(0.2); per-block score matmuls (att as rhs);
  ACT exp into the [w|exp] tile; DVE w = xl*exp; per-block aggregation
  matmul with rhs=[w|exp] yields out and den together; normalize + bias
  (+relu) and store.
"""
import numpy as np

N = 50000
E = 800000
F = 128
HID = 64
NEG_SLOPE = 0.2
NCORES = 8
NPC = N // NCORES            # 6250
NBLK = 50
NPAD = NBLK * 128            # 6400
TBL = NPAD * NCORES          # 51200
LO_LIM = 32768
HEADS = [2, 2, 2, 1]
WCAP = 126                   # max dst nodes per window (row 127 = trash)
EBCAP = 16                   # max edge blocks (128 slots) per window
GRP = 6                      # windows per gather group
QUEUES = 4                   # SWDGE queues used
BPC = 4                      # blocks per score chunk


def _pad128(n):
    return max(128, (int(n) + 127) // 128 * 128)


def _wrap_idx(flat):
    # [n] -> [128, n//16] int16: slot q at [q%16, q//16], tiled over 8 groups
    n = flat.shape[0]
    w = flat.reshape(n // 16, 16).T.astype(np.int16)
    return np.tile(w, (8, 1))


def _plan(deg_lo, deg_hi):
    """Shared window structure: list of (n0, n1, EL, EH) with EL/EH the
    max-over-cores padded slot counts."""
    windows = []
    n0 = 0
    cl = np.zeros(NCORES, np.int64)
    ch = np.zeros(NCORES, np.int64)
    for v in range(NPC):
        nl = cl + deg_lo[:, v]
        nh = ch + deg_hi[:, v]
        blocks = (_pad128(nl.max()) + _pad128(nh.max())) // 128
        if v > n0 and (v - n0 + 1 > WCAP or blocks > EBCAP):
            windows.append((n0, v, _pad128(cl.max()), _pad128(ch.max())))
            n0 = v
            cl = deg_lo[:, v].copy()
            ch = deg_hi[:, v].copy()
        else:
            cl, ch = nl, nh
    windows.append((n0, NPC, _pad128(cl.max()), _pad128(ch.max())))
    return windows


def _prep(inputs):
    import ml_dtypes
    bf16 = ml_dtypes.bfloat16
    x = np.asarray(inputs["x"], dtype=np.float32)
    ei = np.asarray(inputs["edge_index"]).astype(np.int64)
    src, dst = ei[0], ei[1]

    order = np.argsort(dst, kind="stable")
    src, dst = src[order], dst[order]
    prow = (src // NPC) * NPAD + (src % NPC)
    core_of = dst // NPC
    dloc = dst % NPC
    lo_mask = prow < LO_LIM

    deg_lo = np.zeros((NCORES, NPC), np.int64)
    deg_hi = np.zeros((NCORES, NPC), np.int64)
    np.add.at(deg_lo, (core_of, dloc), lo_mask)
    np.add.at(deg_hi, (core_of, dloc), ~lo_mask)

    windows = _plan(deg_lo, deg_hi)
    NW = len(windows)
    groups = [list(range(g, min(g + GRP, NW))) for g in range(0, NW, GRP)]

    # per-core edge lists per (window, lo/hi), in dst-sorted order
    # edge ranges per (core, node) via cumulative counts
    start_of_core = np.searchsorted(core_of, np.arange(NCORES))
    # edges per core are contiguous; within, sorted by dloc
    ebuf_lo = [[] for _ in range(NCORES)]
    ebuf_hi = [[] for _ in range(NCORES)]
    node_ptr = np.zeros((NCORES, NPC + 1), np.int64)
    for c in range(NCORES):
        s0 = start_of_core[c]
        s1 = start_of_core[c + 1] if c + 1 < NCORES else E
        cnt = np.bincount(dloc[s0:s1], minlength=NPC)
        node_ptr[c, 1:] = np.cumsum(cnt) + s0
        node_ptr[c, 0] = s0

    # layout: per window w: EL slots (lo) then EH slots (hi)
    EBTOT = sum((el + eh) // 128 for (_, _, el, eh) in windows)
    LOTOT = sum(el for (_, _, el, _) in windows)
    HITOT = sum(eh for (_, _, _, eh) in windows)

    idxlo = np.zeros((NCORES, 128, LOTOT // 16), np.int16)
    idxhi = np.zeros((NCORES, 128, HITOT // 16), np.int16)
    dstcol = np.zeros((NCORES, 128, EBTOT), bf16)

    lo_off = 0
    hi_off = 0
    col_off = 0
    meta = []  # per window: (n0, n1, EL, EH, col_off)
    for (n0, n1, el, eh) in windows:
        meta.append((n0, n1, el, eh, col_off))
        for c in range(NCORES):
            s0, s1 = node_ptr[c, n0], node_ptr[c, n1]
            pr = prow[s0:s1]
            dl = dloc[s0:s1] - n0
            m = pr < LO_LIM
            flat_lo = np.zeros(el, np.int64)
            nlo = int(m.sum())
            flat_lo[:nlo] = pr[m]
            flat_lo[nlo:] = flat_lo[max(nlo - 1, 0)]
            flat_hi = np.zeros(eh, np.int64)
            nhi = int((~m).sum())
            flat_hi[:nhi] = pr[~m] - LO_LIM
            flat_hi[nhi:] = flat_hi[max(nhi - 1, 0)]
            idxlo[c, :, lo_off // 16:(lo_off + el) // 16] = _wrap_idx(flat_lo)
            idxhi[c, :, hi_off // 16:(hi_off + eh) // 16] = _wrap_idx(flat_hi)
            rows = np.full(el + eh, 127, np.int64)
            rows[:nlo] = dl[m]
            rows[el:el + nhi] = dl[~m]
            eb = (el + eh) // 128
            dstcol[c, :, col_off:col_off + eb] = rows.reshape(eb, 128).T.astype(bf16)
        lo_off += el
        hi_off += eh
        col_off += (el + eh) // 128

    # x shards, padded + cast
    xsh = np.zeros((NCORES, NPAD, F), bf16)
    xv = x.astype(bf16)
    for c in range(NCORES):
        xsh[c, :NPC] = xv[c * NPC:(c + 1) * NPC]

    wboth = np.zeros((4, F, 256), bf16)
    attm = np.zeros((4, F, 2), bf16)
    bias = np.zeros((4, 1, F), np.float32)
    for L in range(4):
        Wl = np.asarray(inputs[f"Wl{L + 1}"], np.float32)
        Wr = np.asarray(inputs[f"Wr{L + 1}"], np.float32)
        att = np.asarray(inputs[f"att{L + 1}"], np.float32)
        b = np.asarray(inputs[f"b{L + 1}"], np.float32)
        fo = Wl.shape[1]
        wboth[L, :, :fo] = Wl.astype(bf16)
        wboth[L, :, 128:128 + fo] = Wr.astype(bf16)
        for h in range(att.shape[0]):
            attm[L, h * HID:(h + 1) * HID, h] = att[h].astype(bf16)
        bias[L, 0, :fo] = b

    ident = np.eye(128, dtype=bf16)
    iota_row = np.arange(128, dtype=np.float32).astype(bf16).reshape(1, 128)
    zeros = np.zeros((NPAD - NPC, F), bf16)

    per_core = []
    for c in range(NCORES):
        per_core.append({
            "x0": xsh[c], "idxlo": idxlo[c], "idxhi": idxhi[c],
            "dstcol": dstcol[c], "wboth": wboth, "attm": attm, "bias": bias,
            "ident": ident, "iota": iota_row, "zeros": zeros,
        })
    plan = (tuple(meta), tuple(tuple(g) for g in groups), LOTOT, HITOT, EBTOT)
    return per_core, plan


# ---------------------------------------------------------------- program

def build_program(plan, n_layers=4, debug=False):
    import concourse.bass as bass
    import concourse.mybir as mybir
    import concourse.tile as tile
    from concourse import bacc

    meta, groups, LOTOT, HITOT, EBTOT = plan
    EBMAX = max((el + eh) // 128 for (_, _, el, eh, _) in meta)
    # per-group slot counts
    gmeta = []
    lo_off = 0
    hi_off = 0
    for g in groups:
        nlo = sum(meta[w][2] for w in g)
        nhi = sum(meta[w][3] for w in g)
        gmeta.append((lo_off, nlo, hi_off, nhi))
        lo_off += nlo
        hi_off += nhi
    NBGMAX = max((nlo + nhi) // 128 for (_, nlo, _, nhi) in gmeta)

    dt = mybir.dt
    nc = bacc.Bacc(None, num_swdge_queues=4)

    x0 = nc.dram_tensor("x0", [NPAD, F], dt.bfloat16, kind="ExternalInput")
    idxlo_d = nc.dram_tensor("idxlo", [128, LOTOT // 16], dt.int16, kind="ExternalInput")
    idxhi_d = nc.dram_tensor("idxhi", [128, HITOT // 16], dt.int16, kind="ExternalInput")
    dstcol_d = nc.dram_tensor("dstcol", [128, EBTOT], dt.bfloat16, kind="ExternalInput")
    wboth_d = nc.dram_tensor("wboth", [4, F, 256], dt.bfloat16, kind="ExternalInput")
    attm_d = nc.dram_tensor("attm", [4, F, 2], dt.bfloat16, kind="ExternalInput")
    bias_d = nc.dram_tensor("bias", [4, 1, F], dt.float32, kind="ExternalInput")
    ident_d = nc.dram_tensor("ident", [128, 128], dt.bfloat16, kind="ExternalInput")
    iota_d = nc.dram_tensor("iota", [1, 128], dt.bfloat16, kind="ExternalInput")
    zeros_d = nc.dram_tensor("zeros", [NPAD - NPC, F], dt.bfloat16, kind="ExternalInput")

    out_d = nc.dram_tensor("out", [NPAD, HID], dt.float32, kind="ExternalOutput")
    if debug:
        eb0 = (meta[0][2] + meta[0][3]) // 128
        nbg0 = (gmeta[0][1] + gmeta[0][3]) // 128
        dbg = {
            "d_xT": nc.dram_tensor("d_xT", [128, NPAD], dt.bfloat16, kind="ExternalOutput"),
            "d_xl": nc.dram_tensor("d_xl", [128, F], dt.bfloat16, kind="ExternalOutput"),
            "d_xr": nc.dram_tensor("d_xr", [128, F], dt.bfloat16, kind="ExternalOutput"),
            "d_xlem": nc.dram_tensor("d_xlem", [128, nbg0 * F], dt.bfloat16, kind="ExternalOutput"),
            "d_bem": nc.dram_tensor("d_bem", [128, eb0 * 128], dt.bfloat16, kind="ExternalOutput"),
            "d_bfm": nc.dram_tensor("d_bfm", [128, eb0 * 128], dt.bfloat16, kind="ExternalOutput"),
            "d_lk": nc.dram_tensor("d_lk", [128, 512], dt.bfloat16, kind="ExternalOutput"),
            "d_sc": nc.dram_tensor("d_sc", [128, 48], dt.float32, kind="ExternalOutput"),
            "d_wexp": nc.dram_tensor("d_wexp", [128, eb0 * 144], dt.bfloat16, kind="ExternalOutput"),
            "d_od": nc.dram_tensor("d_od", [128, 136], dt.float32, kind="ExternalOutput"),
            "d_ps": nc.dram_tensor("d_ps", [128, 512], dt.float32, kind="ExternalOutput"),
            "d_xrw": nc.dram_tensor("d_xrw", [128, F], dt.bfloat16, kind="ExternalOutput"),
        }

    xl_loc = nc.dram_tensor("xl_loc", [NPAD, F], dt.bfloat16)
    xr_loc = nc.dram_tensor("xr_loc", [NPAD, F], dt.bfloat16)
    xl_all = nc.dram_tensor("xl_all", [TBL, F], dt.bfloat16, addr_space="Shared")
    x_cur = nc.dram_tensor("x_cur", [NPAD, F], dt.bfloat16)

    replica = [list(range(NCORES))]
    qrr = [0]  # round-robin queue counter

    with tile.TileContext(nc) as tc:
        cpool = tc.alloc_tile_pool(name="cpool", bufs=1)
        spool = tc.alloc_tile_pool(name="spool", bufs=4)
        gpool = tc.alloc_tile_pool(name="gpool", bufs=3)
        wpool = tc.alloc_tile_pool(name="wpool", bufs=3)
        ppool = tc.alloc_tile_pool(name="ppool", bufs=2, space="PSUM")
        papool = tc.alloc_tile_pool(name="papool", bufs=3, space="PSUM")
        bfpool = tc.alloc_tile_pool(name="bfpool", bufs=1, space="PSUM")

        ident_sb = cpool.tile([128, 128], dt.bfloat16)
        nc.sync.dma_start(out=ident_sb[:], in_=ident_d[:])
        iota_sb = cpool.tile([128, 128], dt.bfloat16)
        nc.sync.dma_start(out=iota_sb[:], in_=iota_d[:].to_broadcast([128, 128]))

        for L in range(n_layers):
            H = HEADS[L]
            C = F // H
            last = L == n_layers - 1
            wb_sb = cpool.tile([F, 256], dt.bfloat16, tag="wb")
            nc.sync.dma_start(out=wb_sb[:], in_=wboth_d[L])
            attm_sb = cpool.tile([F, 2], dt.bfloat16, tag="attm")
            nc.sync.dma_start(out=attm_sb[:], in_=attm_d[L])
            bias_sb = cpool.tile([128, F], dt.float32, tag="bias")
            nc.sync.dma_start(out=bias_sb[:], in_=bias_d[L].to_broadcast([128, F]))

            xin = x0 if L == 0 else x_cur

            # ---- dense: xT via DMA transpose, then xl/xr = x @ [Wl|Wr]
            xT_all = gpool.tile([128, NPAD], dt.bfloat16, tag="xT")
            nc.sync.dma_start(out=xT_all[:], in_=xin[:], transpose=True)
            if debug and L == 0:
                nc.sync.dma_start(out=dbg["d_xT"][:], in_=xT_all[:])
            for blk in range(NBLK):
                plr = ppool.tile([128, 512], dt.float32, tag="schunk", space="PSUM")
                nc.tensor.matmul(out=plr[:, 0:256],
                                 lhsT=xT_all[:, blk * 128:(blk + 1) * 128],
                                 rhs=wb_sb[:], start=True, stop=True)
                xlb = spool.tile([128, F], dt.bfloat16, tag="xlb")
                nc.scalar.activation(out=xlb[:], in_=plr[:, 0:128],
                                     func=mybir.ActivationFunctionType.Copy)
                xrb = spool.tile([128, F], dt.bfloat16, tag="xrb")
                nc.vector.tensor_copy(out=xrb[:], in_=plr[:, 128:256])
                nc.sync.dma_start(out=xl_loc[blk * 128:(blk + 1) * 128, :], in_=xlb[:])
                nc.sync.dma_start(out=xr_loc[blk * 128:(blk + 1) * 128, :], in_=xrb[:])
                if debug and L == 0 and blk == 0:
                    nc.sync.dma_start(out=dbg["d_xl"][:], in_=xlb[:])
                    nc.sync.dma_start(out=dbg["d_xr"][:], in_=xrb[:])

            # ---- allgather xl shards -> full table
            nc.gpsimd.collective_compute(
                "AllGather", mybir.AluOpType.bypass, replica_groups=replica,
                ins=[xl_loc[:]], outs=[xl_all[:]],
            )

            # ---- window groups
            for gi, g in enumerate(groups):
                glo_off, gnlo, ghi_off, gnhi = gmeta[gi]
                nbg_lo = gnlo // 128
                nbg = (gnlo + gnhi) // 128
                ilo = spool.tile([128, max(g2[1] for g2 in gmeta) // 16],
                                 dt.int16, tag="ilo")
                nc.sync.dma_start(out=ilo[:, 0:gnlo // 16],
                                  in_=idxlo_d[:, glo_off // 16:(glo_off + gnlo) // 16])
                ihi = spool.tile([128, max(g2[3] for g2 in gmeta) // 16],
                                 dt.int16, tag="ihi")
                nc.sync.dma_start(out=ihi[:, 0:gnhi // 16],
                                  in_=idxhi_d[:, ghi_off // 16:(ghi_off + gnhi) // 16])

                xl_em = gpool.tile([128, NBGMAX, F], dt.bfloat16, tag="xlem")
                # split lo/hi each into 2 calls so all 4 SWDGE queues (Q7 core
                # pairs + SDMA queue contexts) drain concurrently
                calls = []
                for (base_blk, n_slots, itile, tbl_lo, tbl_hi) in (
                    (0, gnlo, ilo, 0, LO_LIM),
                    (nbg_lo, gnhi, ihi, LO_LIM, TBL),
                ):
                    h1 = (n_slots // 256) * 128
                    for (s0, s1) in ((0, h1), (h1, n_slots)):
                        if s1 > s0:
                            calls.append((base_blk + s0 // 128, s1 - s0, itile,
                                          s0, tbl_lo, tbl_hi))
                for (blk0, nsl, itile, s0, tlo, thi) in calls:
                    nc.gpsimd.dma_gather(
                        out_ap=xl_em[:, blk0:blk0 + nsl // 128, :],
                        in_ap=xl_all[tlo:thi, :],
                        idxs_ap=itile[:, s0 // 16:(s0 + nsl) // 16],
                        num_idxs=nsl, num_idxs_reg=nsl, elem_size=F,
                        transpose=False, single_packet=False,
                        queue_num=(qrr[0] % 4) if QUEUES > 1 else 0)
                    qrr[0] += 1

                blk_lo = 0
                blk_hi = nbg_lo
                for w in g:
                    n0, n1, el, eh, col_off = meta[w]
                    ebw = (el + eh) // 128
                    nlo_b = el // 128
                    nhi_b = eh // 128
                    # window block indices in xl_em
                    wblocks = list(range(blk_lo, blk_lo + nlo_b)) + \
                        list(range(blk_hi, blk_hi + nhi_b))
                    blk_lo += nlo_b
                    blk_hi += nhi_b

                    dcol = spool.tile([128, EBMAX], dt.bfloat16, tag="dcol")
                    nc.sync.dma_start(out=dcol[:, 0:ebw],
                                      in_=dstcol_d[:, col_off:col_off + ebw])
                    xr_win = spool.tile([128, F], dt.bfloat16, tag="xrwin")
                    nc.sync.dma_start(out=xr_win[:], in_=xr_loc[n0:n0 + 128, :])

                    # one-hots
                    b_em = wpool.tile([128, EBMAX, 128], dt.bfloat16, tag="bem")
                    nc.vector.tensor_tensor(
                        out=b_em[:, 0:ebw, :],
                        in0=dcol[:, 0:ebw, None].to_broadcast([128, ebw, 128]),
                        in1=iota_sb[:].unsqueeze(1).to_broadcast([128, ebw, 128]),
                        op=mybir.AluOpType.is_equal)
                    bfm_ps = bfpool.tile([128, EBMAX * 128], dt.bfloat16,
                                         tag="bfm", space="PSUM")
                    for j in range(ebw):
                        nc.tensor.matmul(out=bfm_ps[:, j * 128:(j + 1) * 128],
                                         lhsT=b_em[:, j, :], rhs=ident_sb[:],
                                         is_transpose=True, start=True, stop=True,
                                         skip_group_check=True)
                    b_fm = wpool.tile([128, EBMAX * 128], dt.bfloat16, tag="bfmsb")
                    nc.scalar.activation(out=b_fm[:, 0:ebw * 128],
                                         in_=bfm_ps[:, 0:ebw * 128],
                                         func=mybir.ActivationFunctionType.Copy)
                    if debug and L == 0 and w == 0:
                        nc.sync.dma_start(
                            out=dbg["d_xlem"][:],
                            in_=xl_em[:, 0:(gnlo + gnhi) // 128, :].rearrange(
                                "p b f -> p (b f)"))
                        nc.sync.dma_start(out=dbg["d_bem"][:],
                                          in_=b_em[:, 0:ebw, :].rearrange(
                                              "p b f -> p (b f)"))
                        nc.sync.dma_start(out=dbg["d_bfm"][:],
                                          in_=b_fm[:, 0:ebw * 128])

                    # scores (+ wexp tile holds w and exp)
                    sd_ps = papool.tile([128, 512], dt.float32, tag="sd",
                                        space="PSUM")
                    wexp = wpool.tile([128, EBMAX, 144], dt.bfloat16, tag="wexp")
                    for q in range((ebw + BPC - 1) // BPC):
                        k0 = q * BPC
                        k1 = min(ebw, k0 + BPC)
                        cw = (k1 - k0) * 128
                        ps = ppool.tile([128, 512], dt.float32, tag="schunk",
                                        space="PSUM")
                        for k in range(k0, k1):
                            # xl^T block via regular matmul (identity rhs),
                            # then xr one-hot expansion accumulates into the
                            # SAME region (proper accumulation group).
                            reg = ps[:, (k - k0) * 128:(k - k0 + 1) * 128]
                            nc.tensor.matmul(
                                out=reg,
                                lhsT=xl_em[:, wblocks[k], :], rhs=ident_sb[:],
                                start=True, stop=False,
                                skip_group_check=True)
                            nc.tensor.matmul(
                                out=reg, lhsT=xr_win[:],
                                rhs=b_fm[:, k * 128:(k + 1) * 128],
                                start=False, stop=True, skip_group_check=True)
                        lk = spool.tile([128, 512], dt.bfloat16, tag="lk")
                        nc.scalar.activation(out=lk[:, 0:cw], in_=ps[:, 0:cw],
                                             func=mybir.ActivationFunctionType.Prelu,
                                             alpha=NEG_SLOPE)
                        if debug and L == 0 and w == 0 and q == 0:
                            nc.sync.dma_start(out=dbg["d_lk"][:, 0:cw],
                                              in_=lk[:, 0:cw])
                            ps_sb = spool.tile([128, 512], dt.float32,
                                               tag="psdump")
                            nc.vector.tensor_copy(out=ps_sb[:, 0:cw],
                                                  in_=ps[:, 0:cw])
                            nc.sync.dma_start(out=dbg["d_ps"][:, 0:cw],
                                              in_=ps_sb[:, 0:cw])
                            nc.sync.dma_start(out=dbg["d_xrw"][:], in_=xr_win[:])
                        for k in range(k0, k1):
                            nc.tensor.matmul(
                                out=sd_ps[:, k * H:k * H + H],
                                lhsT=lk[:, (k - k0) * 128:(k - k0 + 1) * 128],
                                rhs=attm_sb[:, 0:H], start=True, stop=True,
                                skip_group_check=True)

                    # exp into wexp[:, :, 128:128+H]
                    nc.scalar.activation(
                        out=wexp[:, 0:ebw, 128:128 + H],
                        in_=sd_ps[:, 0:ebw * H].rearrange("p (b h) -> p b h", h=H),
                        func=mybir.ActivationFunctionType.Exp)

                    # w = xl * exp  (two ops: lo block range, hi block range)
                    for (b0, nb) in ((0, nlo_b), (nlo_b, nhi_b)):
                        if nb == 0:
                            continue
                        xsl = xl_em[:, wblocks[b0]:wblocks[b0] + nb, :]
                        nc.vector.tensor_tensor(
                            out=wexp[:, b0:b0 + nb, 0:128].rearrange(
                                "p b (h c) -> p b h c", h=H),
                            in0=xsl.rearrange("p b (h c) -> p b h c", h=H),
                            in1=wexp[:, b0:b0 + nb, 128:128 + H, None
                                     ].to_broadcast([128, nb, H, C]),
                            op=mybir.AluOpType.mult)

                    if debug and L == 0 and w == 0:
                        sc_sb = spool.tile([128, 48], dt.float32, tag="scdump")
                        nc.vector.tensor_copy(out=sc_sb[:, 0:ebw * H],
                                              in_=sd_ps[:, 0:ebw * H])
                        nc.sync.dma_start(out=dbg["d_sc"][:, 0:ebw * H],
                                          in_=sc_sb[:, 0:ebw * H])

                    # aggregation: out | den
                    od_ps = sd_ps[:, 256:256 + F + H]
                    for j in range(ebw):
                        nc.tensor.matmul(out=od_ps[:],
                                         lhsT=b_em[:, j, :],
                                         rhs=wexp[:, j, 0:F + H],
                                         start=(j == 0), stop=(j == ebw - 1),
                                         skip_group_check=True)
                    if debug and L == 0 and w == 0:
                        nc.sync.dma_start(out=dbg["d_wexp"][:],
                                          in_=wexp[:, 0:ebw, :].rearrange(
                                              "p b f -> p (b f)"))
                        od_sb = spool.tile([128, 136], dt.float32, tag="oddump")
                        nc.vector.tensor_copy(out=od_sb[:, 0:F + H],
                                              in_=od_ps[:])
                        nc.sync.dma_start(out=dbg["d_od"][:, 0:F + H],
                                          in_=od_sb[:, 0:F + H])

                    # normalize + bias (+ relu)
                    den = spool.tile([128, 2], dt.float32, tag="den")
                    nc.vector.tensor_scalar_add(den[:, 0:H], od_ps[:, F:F + H],
                                                1e-20)
                    rec = spool.tile([128, 2], dt.float32, tag="rec")
                    nc.vector.reciprocal(rec[:, 0:H], den[:, 0:H])
                    xo = spool.tile([128, F], dt.float32, tag="xo")
                    for h in range(H):
                        nc.scalar.activation(out=xo[:, h * C:(h + 1) * C],
                                             in_=od_ps[:, h * C:(h + 1) * C],
                                             func=mybir.ActivationFunctionType.Copy,
                                             scale=rec[:, h:h + 1])
                    nc.vector.tensor_tensor(out=xo[:], in0=xo[:], in1=bias_sb[:],
                                            op=mybir.AluOpType.add)
                    nw = n1 - n0
                    if not last:
                        xn = spool.tile([128, F], dt.bfloat16, tag="xn")
                        nc.vector.tensor_scalar_max(xn[:], xo[:], 0.0)
                        nc.sync.dma_start(out=x_cur[n0:n1, :], in_=xn[0:nw, :])
                    else:
                        nc.sync.dma_start(out=out_d[n0:n1, :],
                                          in_=xo[0:nw, 0:HID])

            if not last:
                zpad = spool.tile([128, F], dt.bfloat16, tag="zpad")
                nc.sync.dma_start(out=zpad[:], in_=zeros_d[0:128])
                nc.sync.dma_start(out=x_cur[NPC:NPC + 128, :], in_=zpad[:])
                nc.sync.dma_start(out=x_cur[NPC + 128:NPAD, :],
                                  in_=zpad[0:NPAD - NPC - 128, :])

        for p in (bfpool, papool, ppool, wpool, gpool, spool, cpool):
            p.release()

    nc.finalize()
    return nc


# ---------------------------------------------------------------- runner

_CACHE = {}


def _make_runner(nc, in_maps, n_cores):
    import jax
    from jax.sharding import Mesh, PartitionSpec
    from jax.experimental.shard_map import shard_map
    import concourse.mybir as mybir
    from concourse import bass2jax
    from concourse.bass2jax import _bass_exec_p, partition_id_tensor

    bass2jax.install_neuronx_cc_hook()
    partition_name = nc.partition_id_tensor.name if nc.partition_id_tensor else None
    in_names, out_names, out_avals, zero_outs = [], [], [], []
    for alloc in nc.m.functions[0].allocations:
        if not isinstance(alloc, mybir.MemoryLocationSet):
            continue
        name = alloc.memorylocations[0].name
        if alloc.kind == "ExternalInput":
            if name != partition_name:
                in_names.append(name)
        elif alloc.kind == "ExternalOutput":
            shape = tuple(alloc.tensor_shape)
            dtype = mybir.dt.np(alloc.dtype)
            out_names.append(name)
            out_avals.append(jax.core.ShapedArray(shape, dtype))
            zero_outs.append(np.zeros(shape, dtype))
    n_params = len(in_names)
    n_outs = len(out_avals)
    in_names_all = in_names + out_names
    if partition_name is not None:
        in_names_all.append(partition_name)

    def _body(*args):
        operands = list(args)
        if partition_name is not None:
            operands.append(partition_id_tensor())
        outs = _bass_exec_p.bind(
            *operands, out_avals=tuple(out_avals), in_names=tuple(in_names_all),
            out_names=tuple(out_names), lowering_input_output_aliases=(),
            sim_require_finite=False, sim_require_nnan=False, nc=nc)
        return tuple(outs)

    devices = jax.devices()[:n_cores]
    mesh = Mesh(np.asarray(devices), ("core",))
    in_specs = (PartitionSpec("core"),) * (n_params + n_outs)
    out_specs = (PartitionSpec("core"),) * n_outs
    donate = tuple(range(n_params, n_params + n_outs))
    fn = jax.jit(shard_map(_body, mesh=mesh, in_specs=in_specs,
                           out_specs=out_specs, check_rep=False),
                 donate_argnums=donate, keep_unused=True)
    sharding = jax.sharding.NamedSharding(mesh, PartitionSpec("core"))
    concat_in = [
        jax.device_put(
            np.concatenate([np.asarray(in_maps[c][name]) for c in range(n_cores)],
                           axis=0), sharding)
        for name in in_names
    ]
    zero_big = [np.concatenate([z] * n_cores, axis=0) for z in zero_outs]

    def run():
        import jax as _jax
        outs = fn(*concat_in,
                  *[_jax.device_put(z, sharding) for z in zero_big])
        outs = [np.asarray(o) for o in outs]
        per_core = []
        for c in range(n_cores):
            m = {}
            for i, name in enumerate(out_names):
                rows = out_avals[i].shape[0]
                m[name] = outs[i][c * rows:(c + 1) * rows]
            per_core.append(m)
        return per_core

    return run


def kernel(**inputs) -> np.ndarray:
    per_core, plan = _prep(inputs)
    key = plan[0]
    if key not in _CACHE:
        nc = build_program(plan)
        _CACHE[key] = (nc, None)
    nc, runner = _CACHE[key]
    if runner is None:
        runner = _make_runner(nc, per_core, NCORES)
        _CACHE[key] = (nc, runner)
    res = runner()
    out = np.concatenate([res[c]["out"][:NPC] for c in range(NCORES)], axis=0)
    return out.astype(np.float32)
